# revision 11
# baseline (speedup 1.0000x reference)
"""Trainium2 Bass kernel for nn_Block_9397388444369.

Reference semantics (B=2, T=512, C=256, HID=1024):
    transform = (h @ Wt.T) * 0.0  -> attention branch is exactly bp
    x1  = x + bp
    ff  = relu(LN(x1,g2,b2) @ W1.T + bf1) @ W2.T + bf2
    out = x1 + ff

Device computes only the MLP partials; x1/bp/bf2 are added on the host in
fp32 (exact). LayerNorm is folded into the matmuls:

  z[m,t] = rstd[t] * (sum_c x1[t,c] w1t[c,m] - mu[t] s1[m] + sigma[t] bf1[m])

The "-mu s1 + sigma bf1" term rides as a 2-row augmented matmul (lhsT =
[-s1; bf1] delivered as per-partition columns and PE-transposed on device,
rhs = [mu; sigma] from PE-transposing the bn_stats output), so mm1 runs on
RAW host-transposed x and only the last accumulation waits on stats.
rstd > 0 commutes through the ReLU and is applied once at mm2's fp32
output (t = partition dim there).

mm1/mm2 run in fp8 (e4m3, TRN max +-240) with power-of-2 weight scales
S1=S2=1024 and a 1/16 relu rescale, folded into the final per-partition
multiply (rstd/65536) -- exact in binary. DoubleRow perf mode contracts
both k-chunks in one matmul (2 MACs/cell/cycle). Stats read the fp8 row
copy (simulated end-to-end error ~7.5e-3 vs the 2e-2 gate).

DMA: all inputs ride ONE HWDGE queue (Sync) as 4 ordered blobs --
earliest-needed first -- so nothing contends and Scalar is free for its
two activation-table loads. A dozen dummy matmuls on a memset tile keep
the PE busy from kernel start so the HAM clock gate lifts (1.2->2.4 GHz)
before the real matmuls arrive.
"""

import sys

if '/opt/trn_rl_repo' not in sys.path:
    sys.path.insert(0, '/opt/trn_rl_repo')

import ml_dtypes
import numpy as np

import concourse.bass as bass  # noqa: F401
import concourse.tile as tile
from concourse import bacc, mybir
from concourse.bass_utils import run_bass_kernel_spmd

B, T, C = 2, 512, 256
HID = 4 * C
EPS = 1e-5
N_CORES = 8
N_GROUPS = 4                       # row groups
ROWS = (B * T) // N_GROUPS         # 256 rows per core
RT = ROWS // 128                   # 2 row tiles per core
HH = HID // 2                      # 512-wide hidden half per core
KC = C // 128                      # 2 k-subtiles over C
KH = HH // 128                     # 4 m-chunks over the half
N_WARM = 12                        # dummy matmuls to lift the HAM clock gate

F32 = mybir.dt.float32
BF16 = mybir.dt.bfloat16
FP8 = mybir.dt.float8e4
BF16_NP = ml_dtypes.bfloat16
FP8_NP = ml_dtypes.float8_e4m3

S1 = 1024.0                        # w1 scale (power of 2)
S2 = 1024.0                        # w2 scale
SR = 1.0 / 16.0                    # relu output rescale
STOT = S1 * S2 * SR                # folded into the final rstd multiply


def _build_nc():
    nc = bacc.Bacc("TRN2", target_bir_lowering=False, debug=False,
                   num_devices=N_CORES)

    # 4 input blobs, all on the Sync queue, earliest-needed first
    xr8_d = nc.declare_dram_parameter("xr8", [128, RT * C], FP8,
                                      isOutput=False)
    # identity (128) + augw columns (2 per m-chunk)
    aux_d = nc.declare_dram_parameter("aux", [128, 128 + 2 * KH], BF16,
                                      isOutput=False)
    # per k-plane: [xt_k (256) | w1t_k (512)]
    w1x_d = nc.declare_dram_parameter("w1x", [128, KC, 768], FP8,
                                      isOutput=False)
    # w2 DR pairs: plane q of pair j holds m-chunk 2j+q
    w2_d = nc.declare_dram_parameter("w2", [128, 2, (KH // 2) * C], FP8,
                                     isOutput=False)
    y_d = nc.declare_dram_parameter("y", [128, RT * C], BF16, isOutput=True)

    DR = mybir.MatmulPerfMode.DoubleRow

    with tile.TileContext(nc) as tc:
        with (
            tc.tile_pool(name="acts", bufs=1) as acts,
            tc.tile_pool(name="stats", bufs=2) as stats,
            tc.tile_pool(name="ptrans", bufs=2, space="PSUM") as ptrans,
            tc.tile_pool(name="pmm1", bufs=4, space="PSUM") as pmm1,
            tc.tile_pool(name="pmm2", bufs=2, space="PSUM") as pmm2,
        ):
            # ---- input DMAs, one queue, in need-order ----
            xr8_sb = acts.tile([128, RT * C], FP8)
            nc.sync.dma_start(out=xr8_sb, in_=xr8_d.ap())
            aux_sb = acts.tile([128, 128 + 2 * KH], BF16)
            nc.sync.dma_start(out=aux_sb, in_=aux_d.ap())
            w1x_sb = acts.tile([128, KC, 768], FP8)
            nc.sync.dma_start(out=w1x_sb, in_=w1x_d.ap())
            w2_sb = acts.tile([128, 2, (KH // 2) * C], FP8)
            nc.sync.dma_start(out=w2_sb, in_=w2_d.ap())

            eps_t = acts.tile([128, 1], F32)
            nc.vector.memset(eps_t, np.float32(EPS))
            warm_src = acts.tile([128, 256], BF16)
            nc.vector.memset(warm_src, np.float32(0.5))

            ident = aux_sb[:, 0:128]

            # ---- PE warm-up: keep the array busy so HAM unthrottles ----
            for i in range(N_WARM):
                pw = ptrans.tile([128, 256], F32, tag="pt", name=f"pw_{i}")
                nc.tensor.matmul(pw, lhsT=warm_src[:, 0:128], rhs=warm_src,
                                 start=True, stop=True)

            # ---- stats per row tile ----
            # aug_rhs[0,t] = mu[t]; aug_rhs[1,t] = sqrt(var[t]+eps)
            aug_rhs = acts.tile([2, ROWS], BF16)
            rstd_s = []
            stgs = []
            for r in range(RT):
                xr = xr8_sb[:, r * C:(r + 1) * C]
                bn6 = stats.tile([128, 6], F32, tag="bn6")
                nc.vector.bn_stats(out=bn6, in_=xr)
                mv = stats.tile([128, 2], F32, tag="mv")
                nc.vector.bn_aggr(out=mv, in_=bn6)
                sqv = stats.tile([128, 1], F32, tag="sqv")
                nc.scalar.activation(out=sqv, in_=mv[:, 1:2],
                                     func=mybir.ActivationFunctionType.Sqrt,
                                     bias=eps_t, scale=1.0)
                stg = stats.tile([128, 2], BF16, tag="stg")
                nc.vector.tensor_copy(out=stg[:, 0:1], in_=mv[:, 0:1])
                nc.vector.tensor_copy(out=stg[:, 1:2], in_=sqv)
                stgs.append(stg)
                # rstd/STOT for the final scale
                rstd = stats.tile([128, 1], F32, tag="rstd")
                nc.vector.reciprocal(out=rstd, in_=sqv)
                rs = stats.tile([128, 1], F32, tag="rs")
                nc.vector.tensor_scalar_mul(rs, rstd, 1.0 / STOT)
                rstd_s.append(rs)

            # ---- assemble augw [2, HH] from the DMA'd columns (PE idle) ----
            augw_sb = acts.tile([2, HH], BF16)
            for mc in range(KH):
                pa = ptrans.tile([2, 128], BF16, tag="pt", name=f"pa_{mc}")
                nc.tensor.transpose(pa, aux_sb[:, 128 + 2 * mc:130 + 2 * mc],
                                    ident)
                nc.vector.tensor_copy(
                    out=augw_sb[:, mc * 128:(mc + 1) * 128], in_=pa)

            # ---- stat transposes ----
            for r in range(RT):
                pt = ptrans.tile([2, 128], BF16, tag="pt", name=f"pt_{r}")
                nc.tensor.transpose(pt, stgs[r], ident)
                nc.vector.tensor_copy(
                    out=aug_rhs[:, r * 128:(r + 1) * 128], in_=pt)

            # ---- raw mm1 (fp8 DR: both k-chunks in one matmul) ----
            ps1 = [pmm1.tile([128, ROWS], F32, tag=f"ps1_{i}", bufs=1,
                             name=f"ps1_{i}")
                   for i in range(KH)]
            for mc in range(KH):
                nc.tensor.matmul(
                    ps1[mc],
                    lhsT=w1x_sb[:, :, 256 + mc * 128:256 + (mc + 1) * 128],
                    rhs=w1x_sb[:, :, 0:256],
                    start=True, stop=False,
                    perf_mode=DR,
                )

            # ---- aug matmul (bf16) + relu (alternating engines) ----
            relu1 = acts.tile([128, KH, ROWS], FP8)
            for mc in range(KH):
                nc.tensor.matmul(
                    ps1[mc],
                    lhsT=augw_sb[:, mc * 128:(mc + 1) * 128],
                    rhs=aug_rhs,
                    start=False, stop=True,
                )
                if mc % 2 == 0:
                    nc.scalar.activation(
                        out=relu1[:, mc, :], in_=ps1[mc],
                        func=mybir.ActivationFunctionType.Relu,
                        bias=0.0, scale=float(SR))
                else:
                    nc.vector.tensor_scalar(
                        out=relu1[:, mc, :], in0=ps1[mc],
                        scalar1=0.0, scalar2=float(SR),
                        op0=mybir.AluOpType.max,
                        op1=mybir.AluOpType.mult)

            # ---- mm2 (fp8 DR) + final rstd/STOT scale per row tile ----
            y_sb = acts.tile([128, RT, C], BF16)
            for r in range(RT):
                po = pmm2.tile([128, C], F32)
                for j in range(KH // 2):
                    nc.tensor.matmul(
                        po,
                        lhsT=relu1[:, 2 * j:2 * j + 2, r * 128:(r + 1) * 128],
                        rhs=w2_sb[:, :, j * C:(j + 1) * C],
                        start=(j == 0), stop=(j == KH // 2 - 1),
                        perf_mode=DR,
                    )
                if r == 0:
                    nc.vector.tensor_scalar_mul(y_sb[:, 0, :], po, rstd_s[0])
                    nc.sync.dma_start(out=y_d.ap()[:, :C], in_=y_sb[:, 0, :])
                else:
                    nc.scalar.activation(
                        out=y_sb[:, 1, :], in_=po,
                        func=mybir.ActivationFunctionType.Copy,
                        bias=0.0, scale=rstd_s[1])
                    nc.scalar.dma_start(out=y_d.ap()[:, C:],
                                        in_=y_sb[:, 1, :])

    nc.finalize()
    return nc


_NC_CACHE = None


def _get_nc():
    global _NC_CACHE
    if _NC_CACHE is None:
        _NC_CACHE = _build_nc()
    return _NC_CACHE


def _q8(a, scale):
    s = np.asarray(a, dtype=np.float64) * scale
    s = np.clip(s, -240.0, 240.0)
    return s.astype(np.float32).astype(FP8_NP)


def _pack_inputs(x, bp, g2, b2, W1, bf1, W2):
    x1 = (np.asarray(x, dtype=np.float64).reshape(B * T, C)
          + np.asarray(bp, dtype=np.float64))
    x1_f32 = x1.astype(np.float32)
    x1_f8 = x1_f32.astype(BF16_NP).astype(np.float32).astype(FP8_NP)

    w1t_eff = (np.asarray(W1).astype(np.float64).T
               * np.asarray(g2).astype(np.float64)[:, None])      # [C, HID]
    w1t_f8 = _q8(w1t_eff, S1)
    bf1_eff = (np.asarray(bf1).astype(np.float64)
               + np.asarray(b2).astype(np.float64)
               @ np.asarray(W1).astype(np.float64).T)             # [HID]
    # aug row0 = -sum_c of the scaled fp8 weights actually used
    s1_scaled = w1t_f8.astype(np.float64).sum(axis=0)             # S1-scaled
    aug0 = (-s1_scaled).astype(np.float32).astype(BF16_NP)
    aug1 = (bf1_eff * S1).astype(np.float32).astype(BF16_NP)
    w2t_f8 = _q8(np.asarray(W2, dtype=np.float64).T, S2)          # [HID, C]
    ident = np.eye(128, dtype=np.float32).astype(BF16_NP)

    in_maps = []
    for c in range(N_CORES):
        g, hf = c // 2, c % 2
        xg_f8 = x1_f8[g * ROWS:(g + 1) * ROWS]                    # [256, C]

        xr8 = np.empty((128, RT * C), dtype=FP8_NP)
        for r in range(RT):
            xr8[:, r * C:(r + 1) * C] = xg_f8[r * 128:(r + 1) * 128, :]

        aux = np.empty((128, 128 + 2 * KH), dtype=BF16_NP)
        aux[:, 0:128] = ident
        for mc in range(KH):
            sl = slice(hf * HH + mc * 128, hf * HH + (mc + 1) * 128)
            aux[:, 128 + 2 * mc] = aug0[sl]
            aux[:, 129 + 2 * mc] = aug1[sl]

        w1x = np.empty((128, KC, 768), dtype=FP8_NP)
        w1h = w1t_f8[:, hf * HH:(hf + 1) * HH]                    # [C, HH]
        for k in range(KC):
            w1x[:, k, 0:256] = xg_f8[:, k * 128:(k + 1) * 128].T
            w1x[:, k, 256:768] = w1h[k * 128:(k + 1) * 128, :]

        w2 = np.empty((128, 2, (KH // 2) * C), dtype=FP8_NP)
        w2h = w2t_f8[hf * HH:(hf + 1) * HH]                       # [HH, C]
        for j in range(KH // 2):
            for q in range(2):
                mc = 2 * j + q
                w2[:, q, j * C:(j + 1) * C] = \
                    w2h[mc * 128:(mc + 1) * 128, :]

        in_maps.append({"xr8": xr8, "aux": aux, "w1x": w1x, "w2": w2})
    return in_maps, x1_f32


def _make_in_maps(x, bp, g2, b2, W1, bf1, W2):
    in_maps, _ = _pack_inputs(x, bp, g2, b2, W1, bf1, W2)
    return in_maps


def kernel(x, Wt, Wp, bp, g1, b1, g2, b2, W1, bf1, W2, bf2):
    in_maps, x1_f32 = _pack_inputs(x, bp, g2, b2, W1, bf1, W2)
    nc = _get_nc()
    res = run_bass_kernel_spmd(nc, in_maps, list(range(N_CORES)))

    out = x1_f32.copy()                                       # residual x+bp
    for g in range(N_GROUPS):
        for hf in range(2):
            y = np.asarray(res.results[2 * g + hf]["y"]).astype(np.float32)
            for r in range(RT):
                out[g * ROWS + r * 128:g * ROWS + (r + 1) * 128, :] += \
                    y[:, r * C:(r + 1) * C]
    out = out + np.asarray(bf2, dtype=np.float32)
    return out.reshape(B, T, C).astype(np.float32)
